# revision 18
# baseline (speedup 1.0000x reference)
"""Multi-head causal attention with RoPE on 8 trn2 NeuronCores.

Sharding: core c in 0..7 handles batch b = c//4 and head group g = c%4
(4 of the 16 heads).  Q/K/V projections are column-parallel on the head
dim, attention is head-parallel, and the output projection is
row-parallel on Wo; the 4 partial [S, D] outputs per batch are summed on
the host during unsharding.

On-device layout is feature-major ("transposed") for Q/K/O so every
matmul contraction runs over the partition dim:
  - Q^T, K^T: [256, S] fp32r in SBUF, RoPE applied in feature-major form
    via a stream_shuffle pair trick (head features pre-permuted on the
    host so RoPE partners sit 16 partitions apart inside a 32-quadrant).
  - scores are computed transposed (S^T = K^T.T @ Q^T blocks), causal
    blocks only, with the diagonal triangle masked by adding -1e33
    before the exp.
  - P^T @ [V|1] yields both O^T and the softmax denominator row in one
    accumulation group; normalization multiplies by a broadcast
    reciprocal.
All matmuls run in float32r (full-rate fp32 mode, moving dim >= 256).
"""

import os
import sys

for _p in ("/opt/trn_rl_repo", "/root/.axon_site/_ro/trn_rl_repo"):
    if os.path.isdir(_p) and _p not in sys.path:
        sys.path.insert(0, _p)

import numpy as np

import concourse.bacc as bacc
import concourse.mybir as mybir
from concourse.tile import TileContext
from concourse.bass_utils import run_bass_kernel_spmd

F32 = mybir.dt.float32
F32R = mybir.dt.float32r
I32 = mybir.dt.int32
AF = mybir.ActivationFunctionType
ALU = mybir.AluOpType

B = 2
S = 2048
D = 1024
NH = 16
DK = 64
THETA = 10000.0
NCORES = 8
HPC = 4            # heads per core
LF = HPC * DK      # 256 local features per core
SBLK = 512         # seq block for moving dims
NSB = S // SBLK    # 4
NCT = D // 128     # 8 contraction tiles for projections
VW = DK + 1        # 65: V columns + ones column
SCALE = 1.0 / np.sqrt(DK)

_CACHE = {}


def _feature_perm():
    """Per-head permutation of the 64 head features.

    New index k = 32*u + j maps to old feature 2*(16u + j) for j < 16
    (the RoPE 'x1' half) and 2*(16u + j - 16) + 1 for j >= 16 ('x2').
    Partners then sit 16 partitions apart within a 32-partition quadrant,
    reachable by stream_shuffle.
    """
    perm = np.zeros(DK, dtype=np.int64)
    for k in range(DK):
        u, j = divmod(k, 32)
        if j < 16:
            perm[k] = 2 * (16 * u + j)
        else:
            perm[k] = 2 * (16 * u + j - 16) + 1
    return perm


def _freq_tables():
    """invf [128,1]: |inv_freq|/(2*pi) per partition; sinsc [128,1]: 2*pi*sgn."""
    invf = np.zeros((128, 1), np.float32)
    sinsc = np.zeros((128, 1), np.float32)
    for p in range(128):
        u = (p // 32) % 2
        j = p % 32
        i = 16 * u + (j % 16)
        f = THETA ** (-2.0 * i / DK)
        invf[p] = f / (2 * np.pi)
        sinsc[p] = 2 * np.pi * (1.0 if j < 16 else -1.0)
    return invf, sinsc


def _build(debug=False):
    nc = bacc.Bacc()

    xT_d = nc.dram_tensor("xT", [D, S], F32R, kind="ExternalInput")
    wq_d = nc.dram_tensor("wqT", [D, LF], F32R, kind="ExternalInput")
    wk_d = nc.dram_tensor("wkT", [D, LF], F32R, kind="ExternalInput")
    wv_d = nc.dram_tensor("wvT", [D, LF], F32R, kind="ExternalInput")
    wo_d = nc.dram_tensor("woT", [LF, D], F32R, kind="ExternalInput")
    pos_d = nc.dram_tensor("pos", [1, S], I32, kind="ExternalInput")
    invf_d = nc.dram_tensor("invf", [128, 1], F32, kind="ExternalInput")
    sinsc_d = nc.dram_tensor("sinsc", [128, 1], F32, kind="ExternalInput")
    trim_d = nc.dram_tensor("trimask", [128, 256], F32, kind="ExternalInput")
    y_d = nc.dram_tensor("y", [S, D], F32, kind="ExternalOutput")
    if debug:
        dbg = {n: nc.dram_tensor(n, shp, dt, kind="ExternalOutput")
               for n, shp, dt in [("d_qt0", [128, S], F32R), ("d_kt0", [128, S], F32R),
                                  ("d_vv0", [128, HPC * VW], F32R),
                                  ("d_ot0", [128, S], F32R),
                                  ("d_cos", [128, S], F32), ("d_sin", [128, S], F32),
                                  ("d_sc", [128, SBLK], F32), ("d_pt", [128, SBLK], F32R),
                                  ("d_pv", [VW, SBLK], F32)]}

    shuf_mask = list(range(16, 32)) + list(range(16))

    with TileContext(nc) as tc:
        with tc.tile_pool(name="persist", bufs=1) as pp:
            invf = pp.tile([128, 1], F32)
            sinsc = pp.tile([128, 1], F32)
            trim = pp.tile([128, 256], F32)
            nc.sync.dma_start(out=invf, in_=invf_d[:, :])
            nc.sync.dma_start(out=sinsc, in_=sinsc_d[:, :])

            cos_t = pp.tile([128, S], F32)
            sin_t = pp.tile([128, S], F32)
            qt = [pp.tile([128, S], F32R, name=f"qt{i}") for i in range(2)]
            kt = [pp.tile([128, S], F32R, name=f"kt{i}") for i in range(2)]
            ot = [pp.tile([128, S], F32R, name=f"ot{i}") for i in range(2)]
            vv = [pp.tile([128, HPC * VW], F32R, name=f"vv{i}")
                  for i in range(S // 128)]

            # --- RoPE tables: red = ang - round(ang) in [-0.5, 0.5] turns.
            # Emitted first: the rope of the first projection blocks waits on
            # these, so their DMA + DVE chain must clear early.
            with tc.tile_pool(name="setup", bufs=1) as sp:
                pos_i = sp.tile([128, S], I32)
                nc.sync.dma_start(out=pos_i, in_=pos_d.ap().partition_broadcast(128))
                pos_f = sp.tile([128, S], F32)
                nc.vector.tensor_copy(pos_f, pos_i)
                ang = sp.tile([128, S], F32)
                nc.vector.tensor_scalar(ang, pos_f, invf, None, ALU.mult)
                ki = sp.tile([128, S], I32)
                kf = sp.tile([128, S], F32)
                red = sp.tile([128, S], F32)
                nc.vector.tensor_copy(ki, ang)
                nc.vector.tensor_copy(kf, ki)
                nc.vector.tensor_tensor(red, ang, kf, ALU.subtract)
                # sin table with per-partition sign folded into the scale
                nc.scalar.activation(sin_t, red, AF.Sin, scale=sinsc)
                # cos branch on gpsimd so it doesn't serialize behind the sin
                # branch on DVE
                angc = sp.tile([128, S], F32)
                nc.gpsimd.tensor_scalar(angc, ang, 0.25, None, ALU.add)
                ki2 = sp.tile([128, S], I32)
                kf2 = sp.tile([128, S], F32)
                redc = sp.tile([128, S], F32)
                nc.gpsimd.tensor_copy(ki2, angc)
                nc.gpsimd.tensor_copy(kf2, ki2)
                nc.gpsimd.tensor_tensor(redc, angc, kf2, ALU.subtract)
                nc.scalar.activation(cos_t, redc, AF.Sin, scale=float(2 * np.pi))

            wq_sb = pp.tile([128, NCT * LF], F32R)
            wk_sb = pp.tile([128, NCT * LF], F32R)
            wv_sb = pp.tile([128, NCT * LF], F32R)
            wo_sb = pp.tile([128, 2 * D], F32R)
            for ci in range(NCT):
                nc.sync.dma_start(out=wq_sb[:, LF * ci:LF * (ci + 1)],
                                  in_=wq_d[128 * ci:128 * (ci + 1), :])
            for ci in range(NCT):
                nc.sync.dma_start(out=wk_sb[:, LF * ci:LF * (ci + 1)],
                                  in_=wk_d[128 * ci:128 * (ci + 1), :])
            for ci in range(NCT):
                nc.sync.dma_start(out=wv_sb[:, LF * ci:LF * (ci + 1)],
                                  in_=wv_d[128 * ci:128 * (ci + 1), :])
            nc.sync.dma_start(out=trim, in_=trim_d[:, :])
            for ci in range(2):
                nc.sync.dma_start(out=wo_sb[:, D * ci:D * (ci + 1)],
                                  in_=wo_d[128 * ci:128 * (ci + 1), :])

            # ones column per head block: makes P^T @ [V|1] emit Z rows
            # (memset can't write f32r -> stage in f32 and cast-copy)
            ones4 = pp.tile([128, HPC], F32)
            nc.vector.memset(ones4, 1.0)
            for i in range(S // 128):
                nc.vector.tensor_copy(
                    vv[i].rearrange("p (h x) -> p h x", x=VW)[:, :, DK:DK + 1],
                    ones4.rearrange("p (h x) -> p h x", x=1))

            # ---------------- projections ----------------
            # One psum pool for the whole kernel (no phase barrier):
            #   tag "big": [128, 1024] x2 = 4 banks (qk proj + paired scores)
            #   tag "mid": [VW, 512]  x2 = 2 banks (v proj + pv accumulators)
            #   tag "y":   [128, 512] x2 = 2 banks
            with tc.tile_pool(name="xp", bufs=12) as xp, \
                 tc.tile_pool(name="rope", bufs=2) as rp, \
                 tc.tile_pool(name="pt", bufs=3) as ptp, \
                 tc.tile_pool(name="nrm", bufs=2) as nrm, \
                 tc.tile_pool(name="yo", bufs=4) as yop, \
                 tc.tile_pool(name="zd", bufs=4, space="DRAM") as zdp, \
                 tc.tile_pool(name="ps", bufs=1, space="PSUM") as psp:
                for sb in range(NSB):
                    s0 = SBLK * sb
                    xc = []
                    for ci in range(NCT):
                        t = xp.tile([128, SBLK], F32R, tag="xc", name=f"xc{sb}_{ci}")
                        nc.sync.dma_start(
                            out=t, in_=xT_d[128 * ci:128 * (ci + 1), s0:s0 + SBLK])
                        xc.append(t)
                    for w_sb, dst in ((wq_sb, qt), (wk_sb, kt)):
                        for qf in range(2):
                            ps = psp.tile([128, 2 * SBLK], F32, tag="big", bufs=2,
                                          name=f"prj{sb}_{qf}")
                            for ci in range(NCT):
                                nc.tensor.matmul(
                                    ps[:, 0:SBLK],
                                    w_sb[:, LF * ci + 128 * qf:LF * ci + 128 * (qf + 1)],
                                    xc[ci],
                                    start=(ci == 0), stop=(ci == NCT - 1))
                            a_t = rp.tile([128, SBLK], F32, tag="ra")
                            b_t = rp.tile([128, SBLK], F32, tag="rb")
                            s_t = rp.tile([128, SBLK], F32, tag="rs")
                            nc.vector.tensor_tensor(
                                a_t, ps[:, 0:SBLK], cos_t[:, s0:s0 + SBLK], ALU.mult)
                            nc.vector.tensor_tensor(
                                b_t, ps[:, 0:SBLK], sin_t[:, s0:s0 + SBLK], ALU.mult)
                            nc.vector.stream_shuffle(s_t, b_t, shuf_mask)
                            nc.vector.tensor_tensor(
                                dst[qf][:, s0:s0 + SBLK], a_t, s_t, ALU.add)
                    for st in range(4):
                        ps = psp.tile([128, SBLK], F32, tag="mid", bufs=4,
                                      name=f"vprj{sb}_{st}")
                        for ci in range(NCT):
                            nc.tensor.matmul(
                                ps[:, 0:LF],
                                xc[ci][:, 128 * st:128 * (st + 1)],
                                wv_sb[:, LF * ci:LF * (ci + 1)],
                                start=(ci == 0), stop=(ci == NCT - 1))
                        vt = vv[4 * sb + st]
                        nc.vector.tensor_copy(
                            vt.rearrange("p (h x) -> p h x", x=VW)[:, :, 0:DK],
                            ps[:, 0:LF].rearrange("p (h x) -> p h x", x=DK))

                # ------------- attention + output projection -------------
                for qb in range(NSB):
                    q0 = SBLK * qb
                    nkb = 4 * qb + 4
                    for hp in range(2):
                        pv_ps = [psp.tile([VW, SBLK], F32, tag="mid", bufs=4,
                                          name=f"pv{qb}_{hp}_{hh}")
                                 for hh in range(2)]
                        for kb in range(nkb):
                            r = kb - 4 * qb
                            qlo = 128 * r if r >= 0 else 0
                            sc = psp.tile([128, 2 * SBLK], F32, tag="big", bufs=2,
                                          name=f"sc{qb}_{hp}_{kb}")
                            for hh in range(2):
                                bp = 64 * hh
                                nc.tensor.matmul(
                                    sc[:, SBLK * hh + qlo:SBLK * (hh + 1)],
                                    kt[hp][bp:bp + DK, 128 * kb:128 * (kb + 1)],
                                    qt[hp][bp:bp + DK, q0 + qlo:q0 + SBLK],
                                    start=True, stop=True, skip_group_check=True)
                            scv = sc.rearrange("p (h x) -> p h x", x=SBLK)
                            if r >= 0:
                                nc.vector.tensor_tensor(
                                    scv[:, :, qlo:qlo + 128],
                                    scv[:, :, qlo:qlo + 128],
                                    trim.rearrange("p (h x) -> p h x", x=128),
                                    ALU.add)
                            pt = ptp.tile([128, 2 * SBLK], F32R, tag="pt")
                            ptv = pt.rearrange("p (h x) -> p h x", x=SBLK)
                            nc.scalar.activation(
                                ptv[:, :, qlo:SBLK], scv[:, :, qlo:SBLK], AF.Exp,
                                scale=float(SCALE))
                            if debug and qb == 0 and hp == 0 and kb == 0:
                                dsc = nrm.tile([128, SBLK], F32, name="dsc", bufs=1)
                                nc.vector.tensor_copy(dsc, sc[:, 0:SBLK])
                                nc.sync.dma_start(out=dbg["d_sc"].ap(), in_=dsc)
                                nc.sync.dma_start(out=dbg["d_pt"].ap(),
                                                  in_=pt[:, 0:SBLK])
                            for hh in range(2):
                                lh = 2 * hp + hh
                                nc.tensor.matmul(
                                    pv_ps[hh][:, qlo:SBLK],
                                    vv[kb][:, VW * lh:VW * (lh + 1)],
                                    pt[:, SBLK * hh + qlo:SBLK * (hh + 1)],
                                    start=(kb == 0), stop=(kb == nkb - 1),
                                    skip_group_check=True)
                        if debug and qb == 0 and hp == 0:
                            dpv = nrm.tile([VW, SBLK], F32, name="dpv", bufs=1)
                            nc.vector.tensor_copy(dpv, pv_ps[0])
                            nc.sync.dma_start(out=dbg["d_pv"].ap(), in_=dpv)
                        for hh in range(2):
                            # Z row lives at psum partition 64. Engine lanes
                            # can't shift partitions, so: ACT-copy Z at base 64,
                            # reciprocal in place, then broadcast to partitions
                            # 0..63 by bouncing through DRAM (DRAM-source DMAs
                            # allow a zero partition step; SBUF ones don't).
                            zt = nrm.tile([VW, SBLK], F32, tag="zt")
                            nc.scalar.copy(zt[DK:VW, :], pv_ps[hh][DK:VW, :])
                            zd = zdp.tile([1, SBLK], F32, tag="zd")
                            nc.sync.dma_start(out=zd, in_=zt[DK:VW, :])
                            zb = nrm.tile([DK, SBLK], F32, tag="zb")
                            nc.sync.dma_start(
                                out=zb, in_=zd.partition_broadcast(DK))
                            rb = nrm.tile([DK, SBLK], F32, tag="rbb")
                            nc.vector.reciprocal_approx_fast(out=rb, in_=zb)
                            if hh == 0:
                                nc.vector.tensor_tensor(
                                    ot[hp][0:DK, q0:q0 + SBLK],
                                    pv_ps[hh][0:DK, :], rb, ALU.mult)
                            else:
                                osh = nrm.tile([DK, SBLK], F32R, tag="osh")
                                nc.vector.tensor_tensor(
                                    osh, pv_ps[hh][0:DK, :], rb, ALU.mult)
                                nc.sync.dma_start(
                                    out=ot[hp][DK:2 * DK, q0:q0 + SBLK], in_=osh)
                    for st in range(4):
                        stg = 4 * qb + st
                        for mb in range(2):
                            yps = psp.tile([128, 2 * SBLK], F32, tag="big",
                                           bufs=2, name=f"y{stg}_{mb}")
                            for ci in range(2):
                                nc.tensor.matmul(
                                    yps[:, 0:SBLK],
                                    ot[ci][:, 128 * stg:128 * (stg + 1)],
                                    wo_sb[:, D * ci + SBLK * mb:D * ci + SBLK * (mb + 1)],
                                    start=(ci == 0), stop=(ci == 1))
                            yt = yop.tile([128, SBLK], F32, tag="yt")
                            nc.vector.tensor_copy(yt, yps[:, 0:SBLK])
                            nc.sync.dma_start(
                                out=y_d[128 * stg:128 * (stg + 1),
                                        SBLK * mb:SBLK * (mb + 1)],
                                in_=yt)

            if debug:
                for name, t in (("d_qt0", qt[0]), ("d_kt0", kt[0]), ("d_vv0", vv[0]),
                                ("d_ot0", ot[0]), ("d_cos", cos_t), ("d_sin", sin_t)):
                    nc.sync.dma_start(out=dbg[name].ap(), in_=t)

    nc.finalize()
    return nc


def _prep_inputs(x, token_positions, Wq, Wk, Wv, Wo):
    x = np.asarray(x, dtype=np.float32)
    pos = np.asarray(token_positions, dtype=np.int32).reshape(1, S)
    Wq = np.asarray(Wq, dtype=np.float32)
    Wk = np.asarray(Wk, dtype=np.float32)
    Wv = np.asarray(Wv, dtype=np.float32)
    Wo = np.asarray(Wo, dtype=np.float32)

    perm = _feature_perm()
    invf, sinsc = _freq_tables()
    trimask1 = np.where(np.arange(128)[None, :] >= np.arange(128)[:, None],
                        0.0, -1e33).astype(np.float32)
    trimask = np.concatenate([trimask1, trimask1], axis=1)

    in_maps = []
    for c in range(NCORES):
        b, g = divmod(c, 4)
        rows = slice(LF * g, LF * (g + 1))
        wq_l = Wq[rows].reshape(HPC, DK, D)[:, perm, :].reshape(LF, D)
        wk_l = Wk[rows].reshape(HPC, DK, D)[:, perm, :].reshape(LF, D)
        in_maps.append({
            "xT": np.ascontiguousarray(x[b].T),
            "wqT": np.ascontiguousarray(wq_l.T),
            "wkT": np.ascontiguousarray(wk_l.T),
            "wvT": np.ascontiguousarray(Wv[rows].T),
            "woT": np.ascontiguousarray(Wo[:, rows].T),
            "pos": pos,
            "invf": invf,
            "sinsc": sinsc,
            "trimask": trimask,
        })
    return in_maps


def _run(inputs, trace=False, debug=False, tmpdir=None):
    key = ("nc", debug)
    if key not in _CACHE:
        _CACHE[key] = _build(debug)
    nc = _CACHE[key]
    in_maps = _prep_inputs(**inputs)
    res = run_bass_kernel_spmd(nc, in_maps, list(range(NCORES)), trace=trace,
                               tmpdir=tmpdir)
    y = np.zeros((B, S, D), dtype=np.float32)
    for c in range(NCORES):
        y[c // 4] += res.results[c]["y"]
    return y, res


def kernel(**inputs):
    y, _ = _run(inputs, trace=False)
    return y


# revision 19
# speedup vs baseline: 1.1338x; 1.1338x over previous
"""Multi-head causal attention with RoPE on 8 trn2 NeuronCores.

Sharding: core c in 0..7 handles batch b = c//4 and head group g = c%4
(4 of the 16 heads).  Q/K/V projections are column-parallel on the head
dim, attention is head-parallel, and the output projection is
row-parallel on Wo; the 4 partial [S, D] outputs per batch are summed on
the host during unsharding.

On-device layout is feature-major ("transposed") for Q/K/O so every
matmul contraction runs over the partition dim:
  - Q^T, K^T: [256, S] fp32r in SBUF, RoPE applied in feature-major form
    via a stream_shuffle pair trick (head features pre-permuted on the
    host so RoPE partners sit 16 partitions apart inside a 32-quadrant).
  - scores are computed transposed (S^T = K^T.T @ Q^T blocks), causal
    blocks only, with the diagonal triangle masked by adding -1e33
    before the exp.
  - P^T @ [V|1] yields both O^T and the softmax denominator row in one
    accumulation group; normalization multiplies by a broadcast
    reciprocal.
All matmuls run in float32r (full-rate fp32 mode, moving dim >= 256).
"""

import os
import sys

for _p in ("/opt/trn_rl_repo", "/root/.axon_site/_ro/trn_rl_repo"):
    if os.path.isdir(_p) and _p not in sys.path:
        sys.path.insert(0, _p)

import numpy as np

import concourse.bacc as bacc
import concourse.mybir as mybir
from concourse.tile import TileContext
from concourse.bass_utils import run_bass_kernel_spmd

F32 = mybir.dt.float32
F32R = mybir.dt.float32r
I32 = mybir.dt.int32
AF = mybir.ActivationFunctionType
ALU = mybir.AluOpType

B = 2
S = 2048
D = 1024
NH = 16
DK = 64
THETA = 10000.0
NCORES = 8
HPC = 4            # heads per core
LF = HPC * DK      # 256 local features per core
SBLK = 512         # seq block for moving dims
NSB = S // SBLK    # 4
NCT = D // 128     # 8 contraction tiles for projections
VW = DK + 1        # 65: V columns + ones column
SCALE = 1.0 / np.sqrt(DK)

_CACHE = {}


def _feature_perm():
    """Per-head permutation of the 64 head features.

    New index k = 32*u + j maps to old feature 2*(16u + j) for j < 16
    (the RoPE 'x1' half) and 2*(16u + j - 16) + 1 for j >= 16 ('x2').
    Partners then sit 16 partitions apart within a 32-partition quadrant,
    reachable by stream_shuffle.
    """
    perm = np.zeros(DK, dtype=np.int64)
    for k in range(DK):
        u, j = divmod(k, 32)
        if j < 16:
            perm[k] = 2 * (16 * u + j)
        else:
            perm[k] = 2 * (16 * u + j - 16) + 1
    return perm


def _freq_tables():
    """invf [128,1]: |inv_freq|/(2*pi) per partition; sinsc [128,1]: 2*pi*sgn."""
    invf = np.zeros((128, 1), np.float32)
    sinsc = np.zeros((128, 1), np.float32)
    for p in range(128):
        u = (p // 32) % 2
        j = p % 32
        i = 16 * u + (j % 16)
        f = THETA ** (-2.0 * i / DK)
        invf[p] = f / (2 * np.pi)
        sinsc[p] = 2 * np.pi * (1.0 if j < 16 else -1.0)
    return invf, sinsc


def _build(debug=False):
    nc = bacc.Bacc()

    xT_d = nc.dram_tensor("xT", [D, S], F32R, kind="ExternalInput")
    wq_d = nc.dram_tensor("wqT", [D, LF], F32R, kind="ExternalInput")
    wk_d = nc.dram_tensor("wkT", [D, LF], F32R, kind="ExternalInput")
    wv_d = nc.dram_tensor("wvT", [D, LF], F32R, kind="ExternalInput")
    wo_d = nc.dram_tensor("woT", [LF, D], F32R, kind="ExternalInput")
    pos_d = nc.dram_tensor("pos", [1, S], I32, kind="ExternalInput")
    invf_d = nc.dram_tensor("invf", [128, 1], F32, kind="ExternalInput")
    sinsc_d = nc.dram_tensor("sinsc", [128, 1], F32, kind="ExternalInput")
    trim_d = nc.dram_tensor("trimask", [128, 256], F32, kind="ExternalInput")
    y_d = nc.dram_tensor("y", [S, D], F32, kind="ExternalOutput")
    if debug:
        dbg = {n: nc.dram_tensor(n, shp, dt, kind="ExternalOutput")
               for n, shp, dt in [("d_qt0", [128, S], F32R), ("d_kt0", [128, S], F32R),
                                  ("d_vv0", [128, HPC * VW], F32R),
                                  ("d_ot0", [128, S], F32R),
                                  ("d_cos", [128, S], F32), ("d_sin", [128, S], F32),
                                  ("d_sc", [128, SBLK], F32), ("d_pt", [128, SBLK], F32R),
                                  ("d_pv", [VW, SBLK], F32)]}

    shuf_mask = list(range(16, 32)) + list(range(16))

    with TileContext(nc, pool_alloc_mode="queue") as tc:
        with tc.tile_pool(name="persist", bufs=1) as pp:
            invf = pp.tile([128, 1], F32)
            sinsc = pp.tile([128, 1], F32)
            trim = pp.tile([128, 256], F32)
            nc.sync.dma_start(out=invf, in_=invf_d[:, :])
            nc.sync.dma_start(out=sinsc, in_=sinsc_d[:, :])

            cos_t = pp.tile([128, S], F32)
            sin_t = pp.tile([128, S], F32)
            qt = [pp.tile([128, S], F32R, name=f"qt{i}") for i in range(2)]
            kt = [pp.tile([128, S], F32R, name=f"kt{i}") for i in range(2)]
            ot = [pp.tile([128, S], F32R, name=f"ot{i}") for i in range(2)]
            vv = [pp.tile([128, HPC * VW], F32R, name=f"vv{i}")
                  for i in range(S // 128)]

            # --- RoPE tables: red = ang - round(ang) in [-0.5, 0.5] turns.
            # Emitted first: the rope of the first projection blocks waits on
            # these, so their DMA + DVE chain must clear early.
            with tc.tile_pool(name="setup", bufs=1) as sp:
                pos_i = sp.tile([128, S], I32)
                nc.sync.dma_start(out=pos_i, in_=pos_d.ap().partition_broadcast(128))
                pos_f = sp.tile([128, S], F32)
                nc.vector.tensor_copy(pos_f, pos_i)
                ang = sp.tile([128, S], F32)
                nc.vector.tensor_scalar(ang, pos_f, invf, None, ALU.mult)
                ki = sp.tile([128, S], I32)
                kf = sp.tile([128, S], F32)
                red = sp.tile([128, S], F32)
                nc.vector.tensor_copy(ki, ang)
                nc.vector.tensor_copy(kf, ki)
                nc.vector.tensor_tensor(red, ang, kf, ALU.subtract)
                # sin table with per-partition sign folded into the scale
                nc.scalar.activation(sin_t, red, AF.Sin, scale=sinsc)
                # cos(theta) = sin(2*pi*(ang + 0.25 - round(ang + 0.25)))
                angc = sp.tile([128, S], F32)
                nc.vector.tensor_scalar(angc, ang, 0.25, None, ALU.add)
                nc.vector.tensor_copy(ki, angc)
                nc.vector.tensor_copy(kf, ki)
                nc.vector.tensor_tensor(red, angc, kf, ALU.subtract)
                nc.scalar.activation(cos_t, red, AF.Sin, scale=float(2 * np.pi))

            wq_sb = pp.tile([128, NCT * LF], F32R)
            wk_sb = pp.tile([128, NCT * LF], F32R)
            wv_sb = pp.tile([128, NCT * LF], F32R)
            wo_sb = pp.tile([128, 2 * D], F32R)
            for ci in range(NCT):
                nc.sync.dma_start(out=wq_sb[:, LF * ci:LF * (ci + 1)],
                                  in_=wq_d[128 * ci:128 * (ci + 1), :])
            for ci in range(NCT):
                nc.sync.dma_start(out=wk_sb[:, LF * ci:LF * (ci + 1)],
                                  in_=wk_d[128 * ci:128 * (ci + 1), :])
            for ci in range(NCT):
                nc.sync.dma_start(out=wv_sb[:, LF * ci:LF * (ci + 1)],
                                  in_=wv_d[128 * ci:128 * (ci + 1), :])
            nc.sync.dma_start(out=trim, in_=trim_d[:, :])
            for ci in range(2):
                nc.sync.dma_start(out=wo_sb[:, D * ci:D * (ci + 1)],
                                  in_=wo_d[128 * ci:128 * (ci + 1), :])

            # ones column per head block: makes P^T @ [V|1] emit Z rows
            # (memset can't write f32r -> stage in f32 and cast-copy)
            ones4 = pp.tile([128, HPC], F32)
            nc.vector.memset(ones4, 1.0)
            for i in range(S // 128):
                nc.vector.tensor_copy(
                    vv[i].rearrange("p (h x) -> p h x", x=VW)[:, :, DK:DK + 1],
                    ones4.rearrange("p (h x) -> p h x", x=1))

            # ---------------- projections ----------------
            # One psum pool for the whole kernel (no phase barrier):
            #   tag "big": [128, 1024] x2 = 4 banks (qk proj + paired scores)
            #   tag "mid": [VW, 512]  x2 = 2 banks (v proj + pv accumulators)
            #   tag "y":   [128, 512] x2 = 2 banks
            with tc.tile_pool(name="xp", bufs=12) as xp, \
                 tc.tile_pool(name="rope", bufs=2) as rp, \
                 tc.tile_pool(name="pt", bufs=3) as ptp, \
                 tc.tile_pool(name="nrm", bufs=2) as nrm, \
                 tc.tile_pool(name="yo", bufs=4) as yop, \
                 tc.tile_pool(name="zd", bufs=4, space="DRAM") as zdp, \
                 tc.tile_pool(name="ps", bufs=1, space="PSUM") as psp:
                for sb in range(NSB):
                    s0 = SBLK * sb
                    xc = []
                    for ci in range(NCT):
                        t = xp.tile([128, SBLK], F32R, tag="xc", name=f"xc{sb}_{ci}")
                        nc.sync.dma_start(
                            out=t, in_=xT_d[128 * ci:128 * (ci + 1), s0:s0 + SBLK])
                        xc.append(t)
                    for w_sb, dst in ((wq_sb, qt), (wk_sb, kt)):
                        for qf in range(2):
                            ps = psp.tile([128, 2 * SBLK], F32, tag="big", bufs=2,
                                          name=f"prj{sb}_{qf}")
                            for ci in range(NCT):
                                nc.tensor.matmul(
                                    ps[:, 0:SBLK],
                                    w_sb[:, LF * ci + 128 * qf:LF * ci + 128 * (qf + 1)],
                                    xc[ci],
                                    start=(ci == 0), stop=(ci == NCT - 1))
                            a_t = rp.tile([128, SBLK], F32, tag="ra")
                            b_t = rp.tile([128, SBLK], F32, tag="rb")
                            s_t = rp.tile([128, SBLK], F32, tag="rs")
                            nc.vector.tensor_tensor(
                                a_t, ps[:, 0:SBLK], cos_t[:, s0:s0 + SBLK], ALU.mult)
                            nc.vector.tensor_tensor(
                                b_t, ps[:, 0:SBLK], sin_t[:, s0:s0 + SBLK], ALU.mult)
                            nc.vector.stream_shuffle(s_t, b_t, shuf_mask)
                            nc.vector.tensor_tensor(
                                dst[qf][:, s0:s0 + SBLK], a_t, s_t, ALU.add)
                    for st in range(4):
                        ps = psp.tile([128, SBLK], F32, tag="mid", bufs=4,
                                      name=f"vprj{sb}_{st}")
                        for ci in range(NCT):
                            nc.tensor.matmul(
                                ps[:, 0:LF],
                                xc[ci][:, 128 * st:128 * (st + 1)],
                                wv_sb[:, LF * ci:LF * (ci + 1)],
                                start=(ci == 0), stop=(ci == NCT - 1))
                        vt = vv[4 * sb + st]
                        nc.vector.tensor_copy(
                            vt.rearrange("p (h x) -> p h x", x=VW)[:, :, 0:DK],
                            ps[:, 0:LF].rearrange("p (h x) -> p h x", x=DK))

                # ------------- attention + output projection -------------
                for qb in range(NSB):
                    q0 = SBLK * qb
                    nkb = 4 * qb + 4
                    for hp in range(2):
                        pv_ps = [psp.tile([VW, SBLK], F32, tag="mid", bufs=4,
                                          name=f"pv{qb}_{hp}_{hh}")
                                 for hh in range(2)]
                        for kb in range(nkb):
                            r = kb - 4 * qb
                            qlo = 128 * r if r >= 0 else 0
                            sc = psp.tile([128, 2 * SBLK], F32, tag="big", bufs=2,
                                          name=f"sc{qb}_{hp}_{kb}")
                            for hh in range(2):
                                bp = 64 * hh
                                nc.tensor.matmul(
                                    sc[:, SBLK * hh + qlo:SBLK * (hh + 1)],
                                    kt[hp][bp:bp + DK, 128 * kb:128 * (kb + 1)],
                                    qt[hp][bp:bp + DK, q0 + qlo:q0 + SBLK],
                                    start=True, stop=True, skip_group_check=True)
                            scv = sc.rearrange("p (h x) -> p h x", x=SBLK)
                            if r >= 0:
                                nc.vector.tensor_tensor(
                                    scv[:, :, qlo:qlo + 128],
                                    scv[:, :, qlo:qlo + 128],
                                    trim.rearrange("p (h x) -> p h x", x=128),
                                    ALU.add)
                            pt = ptp.tile([128, 2 * SBLK], F32R, tag="pt")
                            ptv = pt.rearrange("p (h x) -> p h x", x=SBLK)
                            nc.scalar.activation(
                                ptv[:, :, qlo:SBLK], scv[:, :, qlo:SBLK], AF.Exp,
                                scale=float(SCALE))
                            if debug and qb == 0 and hp == 0 and kb == 0:
                                dsc = nrm.tile([128, SBLK], F32, name="dsc", bufs=1)
                                nc.vector.tensor_copy(dsc, sc[:, 0:SBLK])
                                nc.sync.dma_start(out=dbg["d_sc"].ap(), in_=dsc)
                                nc.sync.dma_start(out=dbg["d_pt"].ap(),
                                                  in_=pt[:, 0:SBLK])
                            for hh in range(2):
                                lh = 2 * hp + hh
                                nc.tensor.matmul(
                                    pv_ps[hh][:, qlo:SBLK],
                                    vv[kb][:, VW * lh:VW * (lh + 1)],
                                    pt[:, SBLK * hh + qlo:SBLK * (hh + 1)],
                                    start=(kb == 0), stop=(kb == nkb - 1),
                                    skip_group_check=True)
                        if debug and qb == 0 and hp == 0:
                            dpv = nrm.tile([VW, SBLK], F32, name="dpv", bufs=1)
                            nc.vector.tensor_copy(dpv, pv_ps[0])
                            nc.sync.dma_start(out=dbg["d_pv"].ap(), in_=dpv)
                        for hh in range(2):
                            # Z row lives at psum partition 64. Engine lanes
                            # can't shift partitions, so: ACT-copy Z at base 64,
                            # reciprocal in place, then broadcast to partitions
                            # 0..63 by bouncing through DRAM (DRAM-source DMAs
                            # allow a zero partition step; SBUF ones don't).
                            zt = nrm.tile([VW, SBLK], F32, tag="zt")
                            nc.scalar.copy(zt[DK:VW, :], pv_ps[hh][DK:VW, :])
                            zd = zdp.tile([1, SBLK], F32, tag="zd")
                            nc.sync.dma_start(out=zd, in_=zt[DK:VW, :])
                            zb = nrm.tile([DK, SBLK], F32, tag="zb")
                            nc.sync.dma_start(
                                out=zb, in_=zd.partition_broadcast(DK))
                            rb = nrm.tile([DK, SBLK], F32, tag="rbb")
                            nc.vector.reciprocal_approx_fast(out=rb, in_=zb)
                            if hh == 0:
                                nc.vector.tensor_tensor(
                                    ot[hp][0:DK, q0:q0 + SBLK],
                                    pv_ps[hh][0:DK, :], rb, ALU.mult)
                            else:
                                osh = nrm.tile([DK, SBLK], F32R, tag="osh")
                                nc.vector.tensor_tensor(
                                    osh, pv_ps[hh][0:DK, :], rb, ALU.mult)
                                nc.sync.dma_start(
                                    out=ot[hp][DK:2 * DK, q0:q0 + SBLK], in_=osh)
                    for st in range(4):
                        stg = 4 * qb + st
                        for mb in range(2):
                            yps = psp.tile([128, 2 * SBLK], F32, tag="big",
                                           bufs=2, name=f"y{stg}_{mb}")
                            for ci in range(2):
                                nc.tensor.matmul(
                                    yps[:, 0:SBLK],
                                    ot[ci][:, 128 * stg:128 * (stg + 1)],
                                    wo_sb[:, D * ci + SBLK * mb:D * ci + SBLK * (mb + 1)],
                                    start=(ci == 0), stop=(ci == 1))
                            yt = yop.tile([128, SBLK], F32, tag="yt")
                            nc.vector.tensor_copy(yt, yps[:, 0:SBLK])
                            nc.sync.dma_start(
                                out=y_d[128 * stg:128 * (stg + 1),
                                        SBLK * mb:SBLK * (mb + 1)],
                                in_=yt)

            if debug:
                for name, t in (("d_qt0", qt[0]), ("d_kt0", kt[0]), ("d_vv0", vv[0]),
                                ("d_ot0", ot[0]), ("d_cos", cos_t), ("d_sin", sin_t)):
                    nc.sync.dma_start(out=dbg[name].ap(), in_=t)

    nc.finalize()
    return nc


def _prep_inputs(x, token_positions, Wq, Wk, Wv, Wo):
    x = np.asarray(x, dtype=np.float32)
    pos = np.asarray(token_positions, dtype=np.int32).reshape(1, S)
    Wq = np.asarray(Wq, dtype=np.float32)
    Wk = np.asarray(Wk, dtype=np.float32)
    Wv = np.asarray(Wv, dtype=np.float32)
    Wo = np.asarray(Wo, dtype=np.float32)

    perm = _feature_perm()
    invf, sinsc = _freq_tables()
    trimask1 = np.where(np.arange(128)[None, :] >= np.arange(128)[:, None],
                        0.0, -1e33).astype(np.float32)
    trimask = np.concatenate([trimask1, trimask1], axis=1)

    in_maps = []
    for c in range(NCORES):
        b, g = divmod(c, 4)
        rows = slice(LF * g, LF * (g + 1))
        wq_l = Wq[rows].reshape(HPC, DK, D)[:, perm, :].reshape(LF, D)
        wk_l = Wk[rows].reshape(HPC, DK, D)[:, perm, :].reshape(LF, D)
        in_maps.append({
            "xT": np.ascontiguousarray(x[b].T),
            "wqT": np.ascontiguousarray(wq_l.T),
            "wkT": np.ascontiguousarray(wk_l.T),
            "wvT": np.ascontiguousarray(Wv[rows].T),
            "woT": np.ascontiguousarray(Wo[:, rows].T),
            "pos": pos,
            "invf": invf,
            "sinsc": sinsc,
            "trimask": trimask,
        })
    return in_maps


def _run(inputs, trace=False, debug=False, tmpdir=None):
    key = ("nc", debug)
    if key not in _CACHE:
        _CACHE[key] = _build(debug)
    nc = _CACHE[key]
    in_maps = _prep_inputs(**inputs)
    res = run_bass_kernel_spmd(nc, in_maps, list(range(NCORES)), trace=trace,
                               tmpdir=tmpdir)
    y = np.zeros((B, S, D), dtype=np.float32)
    for c in range(NCORES):
        y[c // 4] += res.results[c]["y"]
    return y, res


def kernel(**inputs):
    y, _ = _run(inputs, trace=False)
    return y
